# revision 53
# baseline (speedup 1.0000x reference)
"""STFT kernel for Trainium2 (8 NeuronCores, batch-parallel).

Computes the equivalent of:
    xp = reflect_pad(x, 512)
    frames[b, f, n] = xp[b, 256*f + n] * window[n]      (f < 1025, n < 1024)
    spec = rfft(frames, axis=-1)                        -> [B, 1025, 513]
    out  = transpose(spec, (0, 2, 1))                   -> [B, 513, 1025] c64

Algorithm (radix-4 decimation-in-frequency over the hop structure):
with n = 256*j + r and k = c + 4*k2 (c = k mod 4), e^{-i*th*k*256*j} =
(-i)^(c*j) depends only on c, so

    spec[f, k] = sum_r e^{-i*th*k*r} * U_c[f, r],
    U_c[f, r]  = sum_j (-i)^(c*j) * P_j[f, r],   P_j[f, r] = w[256j+r] Y[f+j, r]

where Y[g, r] = xp[256*g + r] and th = 2*pi/1024.  The host sends two
HALF-prewindowed copies of the transposed hop matrix per r-half:

    a = w2 * Y      (w2 = w[512+r'] in [0.5, 1], so no f16 subnormals)
    b = w1 * Y(+1)  (w1 = w[256+r'] in [0.5, 1])

so that P2 = a[:, 2:] and P1 = b[:, :] are free column views, and
P0 = (w0/w2) a, P3 = (w3/w1) b[:, 2:] are single tensor-scalar products
with per-partition ratios in [0, 1].  The U-build is then 8 DVE ops per
r-half (all hitting the DVE fast modes; GpSimd is left idle on purpose --
any concurrent GpSimd elementwise op degrades DVE throughput
several-fold):

    P0 = ts(a, w0/w2)            u1rn = P2 - P0 (= -Re U1)
    P3 = ts(b[:,2:], w3/w1)      u1i  = P3 - P1 (= Im U1)
    q  = P0 + P2                 r    = P1 + P3
    u2 = q - r                   u0   = q + r

Each frequency class c is a short TensorE matmul with contraction over r
(256), fp16 operands, fp32 PSUM accumulation:

    re1 = -C1*u1rn - S1*u1i   im1 = -S1*u1rn + C1*u1i   (C = cos, S = -sin)
    re3 = -C3*u1rn + S3*u1i   im3 = -S3*u1rn - C3*u1i
    re2 =  C2*u2              im2 =  S2*u2
    re0 =  C0*u0              im0 =  S0*u0

Nyquist (k=512): re = sum_r (-1)^r u0[f, r].  S0's k2=0 column is
identically zero (im of k=0), so the (-1)^r vector is folded into that
column; the host extracts the nyquist row from im[k=0] and re-zeroes it.

Only frames 0..1023 are computed on device (chunks of 512 matching PSUM
banks); the final frame 1024 is a single rfft done on the host.  Each
(batch, class, chunk) accumulates re/im into a 2-bank PSUM tile
[128, 2, 512] drained by one Activation-engine copy (f32 -> f16 cast) into
an SBUF tile written out densely; the host interleaves the re/im planes to
complex64.  DMA traffic is split across both HWDGE queues (Sync and
Activation): a-tiles/outputs of classes 1,2 on Sync, b-tiles/outputs of
classes 3,0 on Activation, and the last batch drains per-chunk so the two
queues empty in parallel at the tail.

Batch dim (16) is sharded across the 8 cores, 2 batches each; no
cross-device communication.
"""

from contextlib import ExitStack

import numpy as np

import concourse.mybir as mybir
import concourse.tile as tile
from concourse import bacc
from concourse.bass_utils import run_bass_kernel_spmd

NFFT, HOP, PAD = 1024, 256, 512
B, T = 16, 262144
NCORES = 8
BC = B // NCORES                 # batches per core
G = (T + 2 * PAD) // HOP         # 1028 hop blocks per padded row
NF = (T + 2 * PAD - NFFT) // HOP + 1   # 1025 frames
ND = 1024                        # frames computed on device (last one on host)
NW = ND + 2                      # input tile width (shifted views need +2)
KF = NFFT // 2 + 1               # 513 one-sided freqs
CHUNKS = [(0, 512), (512, 512)]
NMAT = 12

_cache = {}

import ml_dtypes
DT16 = mybir.dt.float16
NP16 = np.float16


def _build():
    nc = bacc.Bacc(
        "TRN2", target_bir_lowering=False, debug=False, num_devices=NCORES
    )
    f16 = DT16
    f32 = mybir.dt.float32
    # xab[b, h, 0] = w2*Y, xab[b, h, 1] = w1*Y(+1)  (per r-half h)
    xab_d = nc.dram_tensor("xab", [BC, 2, 2, 128, NW], f16, kind="ExternalInput")
    # mats order: c1 (0..3), c3 (4..7), c0 (8,9), c2 (10,11)
    wm_d = nc.dram_tensor("wm", [128, NMAT, 2, 128], f16, kind="ExternalInput")
    # wsc[p, h] = w0/w2 ratio, wsc[p, 2+h] = w3/w1 ratio
    wsc_d = nc.dram_tensor("wsc", [128, 4], f32, kind="ExternalInput")
    out_d = nc.dram_tensor("out", [BC, 128, 4, 2, ND], f16, kind="ExternalOutput")

    with tile.TileContext(nc) as tc, ExitStack() as ctx:
        consts = ctx.enter_context(tc.tile_pool(name="consts", bufs=1))
        xpool = ctx.enter_context(tc.tile_pool(name="x", bufs=1))
        tpool = ctx.enter_context(tc.tile_pool(name="t", bufs=2))
        upool = ctx.enter_context(tc.tile_pool(name="u", bufs=2))
        opool = ctx.enter_context(tc.tile_pool(name="o", bufs=2))
        ppool = ctx.enter_context(tc.tile_pool(name="psum", bufs=4, space="PSUM"))

        # ---- loads: everything batch-0 needs goes on the (fast) Sync
        # queue in exactly the order the DVE chain consumes it; the
        # Activation queue gets only wsc and the time-gated second half of
        # batch 1, so it never steals bandwidth from the critical path. ----
        xs = {}
        for b in range(BC):
            for h in range(2):
                for t in range(2):
                    xs[(b, h, t)] = xpool.tile([128, NW], f16, name=f"x{b}{h}{t}")
        wsc = consts.tile([128, 4], f32)
        nc.scalar.dma_start(wsc[:], wsc_d.ap())
        nc.sync.dma_start(xs[(0, 0, 0)][:], xab_d.ap()[0, 0, 0])
        wmA = consts.tile([128, 4, 2, 128], f16)
        nc.sync.dma_start(wmA[:], wm_d.ap()[:, 0:4])
        nc.sync.dma_start(xs[(0, 0, 1)][:], xab_d.ap()[0, 0, 1])
        nc.sync.dma_start(xs[(0, 1, 0)][:], xab_d.ap()[0, 1, 0])
        nc.sync.dma_start(xs[(0, 1, 1)][:], xab_d.ap()[0, 1, 1])
        wmB = consts.tile([128, NMAT - 4, 2, 128], f16)
        nc.sync.dma_start(wmB[:], wm_d.ap()[:, 4:NMAT])
        for h in range(2):
            nc.sync.dma_start(xs[(1, h, 0)][:], xab_d.ap()[1, h, 0])
        with tc.tile_wait_until(0.010):
            for h in range(2):
                nc.scalar.dma_start(xs[(1, h, 1)][:], xab_d.ap()[1, h, 1])

        def wmat(mi):
            return wmA[:, mi] if mi < 4 else wmB[:, mi - 4]

        # ---- PE warm-up: dummy matmuls during the otherwise-idle input
        # phase so HAM reaches full clock before the real stream starts ----
        ze = consts.tile([128, 640], f16)
        nc.vector.memset(ze[:], 0.0)
        psw = ppool.tile([128, 2, 512], mybir.dt.float32, name="ps")
        for i in range(10):
            nc.tensor.matmul(
                psw[:, 0, :], ze[:, 0:128], ze[:, 128:640],
                start=(i == 0), stop=(i == 9),
            )

        for b in range(BC):
            # ---- build U per r-half (all DVE, fast ts/tt ops only) ----
            U = {}
            PV = {}
            for h in range(2):
                a = xs[(b, h, 0)]
                bb = xs[(b, h, 1)]
                p0 = tpool.tile([128, ND], f16, name=f"p0{h}")
                nc.vector.tensor_scalar_mul(p0[:], a[:, 0:ND], wsc[:, h : h + 1])
                p3 = tpool.tile([128, ND], f16, name=f"p3{h}")
                nc.vector.tensor_scalar_mul(
                    p3[:], bb[:, 2 : 2 + ND], wsc[:, 2 + h : 3 + h]
                )
                u1rn = upool.tile([128, ND], f16, name=f"u1rn{h}")
                nc.vector.tensor_sub(u1rn[:], a[:, 2 : 2 + ND], p0[:])
                u1i = upool.tile([128, ND], f16, name=f"u1i{h}")
                nc.vector.tensor_sub(u1i[:], p3[:], bb[:, 0:ND])
                U[("u1rn", h)] = u1rn
                U[("u1i", h)] = u1i
                PV[h] = (p0, p3, a, bb)
            QR = {}
            for h in range(2):
                p0, p3, a, bb = PV[h]
                q = upool.tile([128, ND], f16, name=f"q{h}")
                nc.vector.tensor_add(q[:], p0[:], a[:, 2 : 2 + ND])
                r_ = upool.tile([128, ND], f16, name=f"r{h}")
                nc.vector.tensor_add(r_[:], bb[:, 0:ND], p3[:])
                QR[h] = (q, r_)
            for h in range(2):
                q, r_ = QR[h]
                u2 = upool.tile([128, ND], f16, name=f"u2{h}")
                nc.vector.tensor_sub(u2[:], q[:], r_[:])
                U[("u2", h)] = u2
            for h in range(2):
                q, r_ = QR[h]
                u0 = upool.tile([128, ND], f16, name=f"u0{h}")
                nc.vector.tensor_add(u0[:], q[:], r_[:])
                U[("u0", h)] = u0

            # ---- frequency classes; output queues alternate so the two
            # HWDGE queues drain in parallel; the last batch goes per-chunk
            # and the final chunk in small pieces ----
            terms_of = {
                1: ([(0, "u1rn"), (1, "u1i")], [(2, "u1rn"), (3, "u1i")]),
                3: ([(4, "u1rn"), (5, "u1i")], [(6, "u1rn"), (7, "u1i")]),
                2: ([(10, "u2")], [(11, "u2")]),
                0: ([(8, "u0")], [(9, "u0")]),
            }
            qmap = (
                {1: nc.sync, 3: nc.scalar, 2: nc.sync, 0: nc.scalar}
                if b != BC - 1
                else {1: nc.scalar, 3: nc.sync, 2: nc.scalar, 0: nc.sync}
            )
            ots = {
                c: opool.tile([128, 2, ND], f16, name=f"ot{c}")
                for c in (1, 3, 2, 0)
            }

            def emit_mms(c, ci, ps, h):
                f0, fn = CHUNKS[ci]
                for part in (0, 1):
                    terms = terms_of[c][part]
                    for ti, (mi, uname) in enumerate(terms):
                        nc.tensor.matmul(
                            ps[:, part, :fn],
                            wmat(mi)[:, h, :],
                            U[(uname, h)][:, f0 : f0 + fn],
                            start=(h == 0 and ti == 0),
                            stop=(h == 1 and ti == len(terms) - 1),
                        )

            def drain(c, ci, ps):
                f0, fn = CHUNKS[ci]
                qeng = qmap[c]
                ot = ots[c]
                last = b == BC - 1 and c == 0
                # the last class drains via the (idle) DVE so its copies
                # run in parallel with the Activation engine's
                if not (last and ci == 1):
                    if last:
                        nc.vector.tensor_copy(ot[:, :, f0 : f0 + fn], ps[:])
                    else:
                        nc.scalar.copy(ot[:, :, f0 : f0 + fn], ps[:])
                else:
                    # drain the very last unit in small pieces
                    for e0, e1 in ((512, 768), (768, 1024)):
                        nc.vector.tensor_copy(
                            ot[:, :, e0:e1], ps[:, :, e0 - 512 : e1 - 512]
                        )
                        qeng.dma_start(
                            out_d.ap()[b, :, c, :, e0:e1], ot[:, :, e0:e1]
                        )
                if b == BC - 1 and not (last and ci == 1):
                    qeng.dma_start(
                        out_d.ap()[b, :, c, :, f0 : f0 + fn],
                        ot[:, :, f0 : f0 + fn],
                    )
                if b != BC - 1 and ci == 1:
                    qeng.dma_start(out_d.ap()[b, :, c], ots[c][:])

            # c1/c3: all h0 matmuls first (they only need the early-arriving
            # h0 U tensors and fill the PE while DVE builds h1), then the h1
            # matmuls close the four open accumulation groups.
            cc_ps = {}
            for c in (1, 3):
                for ci in range(2):
                    cc_ps[(c, ci)] = ppool.tile(
                        [128, 2, 512], mybir.dt.float32, name="ps"
                    )
                    emit_mms(c, ci, cc_ps[(c, ci)], 0)
            for c in (1, 3):
                for ci in range(2):
                    emit_mms(c, ci, cc_ps[(c, ci)], 1)
                    drain(c, ci, cc_ps[(c, ci)])
            for c, ci in [(2, 0), (2, 1), (0, 0), (0, 1)]:
                ps = ppool.tile([128, 2, 512], mybir.dt.float32, name="ps")
                emit_mms(c, ci, ps, 0)
                emit_mms(c, ci, ps, 1)
                drain(c, ci, ps)
    nc.compile()
    return nc


def _consts(window):
    w = np.asarray(window, np.float64)
    th = 2.0 * np.pi / NFFT
    r = np.arange(256, dtype=np.float64)[:, None]
    k2 = np.arange(128, dtype=np.float64)[None, :]

    def cs(c):
        ang = th * (c + 4.0 * k2) * r
        return np.cos(ang), -np.sin(ang)

    C0, S0 = cs(0)
    C1, S1 = cs(1)
    C2, S2 = cs(2)
    C3, S3 = cs(3)
    S0 = S0.copy()
    S0[:, 0] = (-1.0) ** np.arange(256)  # nyquist row folded into im[k=0]
    mats = [-C1, -S1, -S1, C1, -C3, S3, -S3, -C3, C0, S0, C2, S2]
    # [256(r), 128(k2)] -> [128(p), 2(h), 128], stacked -> [128, NMAT, 2, 128]
    wm = np.stack(
        [m.reshape(2, 128, 128).transpose(1, 0, 2) for m in mats], axis=1
    ).astype(NP16)
    wm = np.ascontiguousarray(wm)

    ws = w.reshape(4, 2, 128)  # ws[j, h, p] = w[256j + 128h + p]
    wsc = np.empty((128, 4), np.float32)
    wsc[:, 0:2] = (ws[0] / ws[2]).T  # w0/w2 per (p, h)
    wsc[:, 2:4] = (ws[3] / ws[1]).T  # w3/w1 per (p, h)
    return wm, wsc, ws


def prep_inputs(x, window):
    """Host-side shard/layout prep: per-core input maps."""
    xp = np.pad(np.asarray(x, np.float32), ((0, 0), (PAD, PAD)), mode="reflect")
    xt = xp.reshape(B, G, HOP).transpose(0, 2, 1).reshape(B, 2, 128, G)
    wm, wsc, ws = _consts(window)
    w2 = ws[2].astype(np.float32)[None, :, :, None]  # [1, 2, 128, 1]
    w1 = ws[1].astype(np.float32)[None, :, :, None]
    xab = np.empty((B, 2, 2, 128, NW), NP16)
    xab[:, :, 0] = (xt[:, :, :, 0:NW] * w2).astype(NP16)
    xab[:, :, 1] = (xt[:, :, :, 1 : 1 + NW] * w1).astype(NP16)
    # host-side rfft of the final frame
    lastspec = np.fft.rfft(
        xp[:, HOP * (NF - 1) :].astype(np.float64) * np.asarray(window, np.float64)
    ).astype(np.complex64)
    _cache["lastspec"] = lastspec
    return [
        {
            "xab": xab[i * BC : (i + 1) * BC],
            "wm": wm,
            "wsc": wsc,
        }
        for i in range(NCORES)
    ]


def get_nc():
    nc = _cache.get("nc")
    if nc is None:
        nc = _build()
        _cache["nc"] = nc
    return nc


def kernel(x, window, _trace=False, _trace_kwargs=None):
    nc = get_nc()
    in_maps = prep_inputs(x, window)
    res = run_bass_kernel_spmd(
        nc, in_maps, list(range(NCORES)), trace=_trace, **(_trace_kwargs or {})
    )
    _cache["last_results"] = res
    # A: [B, 128(k2), 4(c), 2(re/im), ND] f16
    A = np.concatenate([r["out"] for r in res.results], axis=0).astype(np.float32)
    re = A[:, :, :, 0, :].reshape(B, 512, ND)
    im = A[:, :, :, 1, :].reshape(B, 512, ND)
    nyq = im[:, 0, :].copy()  # folded into S0's k2=0 column
    out = np.empty((B, KF, NF), np.complex64)
    out.real[:, :512, :ND] = re
    out.imag[:, :512, :ND] = im
    out.imag[:, 0, :ND] = 0.0
    out.real[:, 512, :ND] = nyq
    out.imag[:, 512, :ND] = 0.0
    out[:, :, ND] = _cache["lastspec"]
    return out
